# revision 8
# baseline (speedup 1.0000x reference)
"""DigitCaps (capsule routing) Trainium2 kernel, v2.

Contract: kernel(x, W) takes the FULL inputs
  x: [512, 32, 8, 6, 6] fp32, W: [1, 10, 1152, 16, 8] fp32
and returns v: [512, 10, 16] fp32, running on 8 NeuronCores with the
batch sharded 8 ways (64 per core) and W replicated.

Math (per reference):
  xr = x.reshape(B, 1152, 8)
  u[b,o,i,w] = sum_s W[o,i,w,s] xr[b,i,s]
  3 routing iterations of: c = softmax_o(beta); s = sum_i c*u;
  v = squash(s); beta += sum_w u*v   (last iteration's beta update is dead)

v2 structure (vs v1 baseline):
  - s-matmuls (iters 2,3) are blocked: per (o, chunk, s-parity) ONE
    matmul with lhsT = W[o, i128, 32*(s//2)+w] (s-blocks padded to 32
    rows so extraction slices are 32-partition aligned) streaming
    y[i, (s', b)] 256 cols; a small DVE tree extracts
    s[w,b] = sum_s psum[32(s//2)+w, (s,b)].  180 matmuls/iter
    instead of 720.
  - softmax (exp/Z/EZ) is emitted per-chunk inside the agreement loop
    so it overlaps the remaining chunks instead of serializing.
  - sqrt in squash via exp(0.5*ln(n2)) -- keeps the scalar engine on
    one activation table set (no ~2.7us ACT_TABLE_LOAD swaps).
  - input + V8D-replication DMAs spread across engine queues.

Layouts (i-partitioned; p is the SBUF partition index):
  XB   [p,c,s,b]            = xr[b, 128c+p, s]                  bf16
  WSWP [p,c,o,q,32(s//2)+w] = W[o, 128c+p, w, s], s = 2*(s//2)+q bf16
                              (cols 32k+16..32k+31 are zero)
  W2   [16s+w, o, i]        = W[o,i,w,s]                        bf16
All heavy compute in bf16 with fp32 PSUM accumulation; routing logits
(A1/beta) in fp32.
"""

import sys

import numpy as np

if "/opt/trn_rl_repo" not in sys.path:
    sys.path.insert(0, "/opt/trn_rl_repo")

import ml_dtypes

import concourse.bass as bass
import concourse.tile as tile
from concourse import bacc, mybir
from concourse.bass_utils import run_bass_kernel_spmd

BF = ml_dtypes.bfloat16
F32 = mybir.dt.float32
BF16 = mybir.dt.bfloat16

B, O, I, OW, S = 512, 10, 1152, 16, 8
NCORES = 8
BL = B // NCORES          # 64 batches per core
C9 = I // 128             # 9 i-chunks
AF = mybir.ActivationFunctionType
ALU = mybir.AluOpType


# ---------------------------------------------------------------------------
# device program
# ---------------------------------------------------------------------------

def _emit(nc, tc, t, ctx):
    """Emit the whole per-core program. `t` holds dram tensor handles."""
    P = ctx.enter_context(tc.tile_pool(name="pers", bufs=1))
    WK = ctx.enter_context(tc.tile_pool(name="work", bufs=2))
    WK1 = ctx.enter_context(tc.tile_pool(name="work1", bufs=1))
    SM = ctx.enter_context(tc.tile_pool(name="small", bufs=1))

    # ---- persistent SBUF tensors -----------------------------------------
    XB = P.tile([128, C9, S, BL], BF16, name="XB")
    WSWP = P.tile([128, C9, O, 2, 128], BF16, name="WSWP")
    W2 = P.tile([128, O, I], BF16, name="W2")
    ID64 = P.tile([64, 64], F32, name="ID64")
    ID16 = P.tile([16, 16], F32, name="ID16")
    V8D = P.tile([128, O, S, BL], BF16, name="V8D")
    A1 = P.tile([128, C9, O, BL], F32, name="A1")      # beta accumulator
    EZ = P.tile([128, C9, O, BL], BF16, name="EZ")     # exp(beta)/Z

    nc.sync.dma_start(XB[:], t["xb"][:])
    nc.scalar.dma_start(WSWP[:], t["wswp"][:])
    nc.gpsimd.dma_start(W2[:], t["w2"][:])
    nc.sync.dma_start(ID64[:], t["id64"][:])
    nc.sync.dma_start(ID16[:], t["id16"][:])
    nc.gpsimd.memset(V8D[:], 0.0)

    # ---- small helpers ----------------------------------------------------
    def squash(s_bT, scale, it):
        """s_bT: [64, O, OW] fp32 SBUF -> v_sb [64, O, OW] fp32.

        v = gamma(n2)*(scale*s), n2 = |scale*s|^2,
        gamma = sqrt(n2)/(1+n2); sqrt via exp(.5*ln) (one ACT table set).
        """
        ssq = SM.tile([64, O, OW], F32, name=f"ssq{it}", tag="ssq")
        nc.scalar.activation(ssq[:], s_bT[:], AF.Square, scale=float(scale))
        n2 = SM.tile([64, O], F32, name=f"n2_{it}", tag="n2")
        nc.vector.reduce_sum(
            n2[:].unsqueeze(2), ssq[:], axis=mybir.AxisListType.X)
        ln2 = SM.tile([64, O], F32, name=f"ln2_{it}", tag="ln2")
        nc.scalar.activation(ln2[:], n2[:], AF.Ln)
        sq = SM.tile([64, O], F32, name=f"sq_{it}", tag="sq")
        nc.scalar.activation(sq[:], ln2[:], AF.Exp, scale=0.5)
        n2p1 = SM.tile([64, O], F32, name=f"n2p1_{it}", tag="n2p1")
        nc.vector.tensor_scalar_add(n2p1[:], n2[:], 1.0)
        rden = SM.tile([64, O], F32, name=f"rden_{it}", tag="rden")
        nc.vector.reciprocal_approx_fast(rden[:], n2p1[:])
        gam = SM.tile([64, O], F32, name=f"gam_{it}", tag="gam")
        nc.vector.tensor_mul(gam[:], sq[:], rden[:])
        if scale != 1.0:
            gam2 = SM.tile([64, O], F32, name=f"gam2_{it}", tag="gam2")
            nc.scalar.mul(gam2[:], gam[:], float(scale))
            gam = gam2
        v_sb = SM.tile([64, O, OW], F32, name=f"v_sb{it}", tag="v_sb")
        nc.vector.tensor_mul(
            v_sb[:], s_bT[:],
            gam[:].unsqueeze(2).broadcast_to([64, O, OW]))
        return v_sb

    def build_v8d(v_sb, it):
        """v_sb [64, O, OW] fp32 -> V8D block-diag (bf16).

        V8D[16s+w, o, s', b] = v[b,o,w] * (s == s')."""
        with tc.tile_pool(name=f"vps_{it}", space="PSUM", bufs=1) as vp:
            vtp = vp.tile([16, O, BL], F32, name=f"vtp_{it}")
            for o in range(O):
                nc.tensor.transpose(vtp[:, o, :], v_sb[:, o, :], ID64[:])
            vT = SM.tile([16, O, BL], BF16, name=f"vT_{it}", tag="vT")
            nc.scalar.copy(vT[:], vtp[:])
        qs = [nc.sync, nc.scalar, nc.gpsimd]
        for s in range(S):
            qs[s % 3].dma_start(V8D[16 * s:16 * s + 16, :, s, :], vT[:])

    def softmax_chunk(c, it):
        """EZ[c] = softmax_o(A1[c]) = exp(A1[c])/sum_o exp(A1[c])."""
        Ech = WK.tile([128, O, BL], BF16, name=f"E_{it}_{c}", tag="Ech")
        nc.scalar.activation(Ech[:], A1[:, c], AF.Exp)
        t5 = WK1.tile([128, 5, BL], F32, name=f"t5_{it}_{c}", tag="t5")
        nc.vector.tensor_add(t5[:], Ech[0:128, 0:5, :], Ech[0:128, 5:10, :])
        u2 = WK1.tile([128, 2, BL], F32, name=f"u2_{it}_{c}", tag="u2")
        nc.vector.tensor_add(u2[:], t5[:, 0:2, :], t5[:, 2:4, :])
        zs = WK1.tile([128, BL], F32, name=f"zs_{it}_{c}", tag="zs")
        nc.vector.tensor_add(zs[:], u2[:, 0, :], u2[:, 1, :])
        nc.vector.tensor_add(zs[:], zs[:], t5[:, 4, :])
        rz = WK1.tile([128, BL], F32, name=f"rz_{it}_{c}", tag="rz")
        nc.vector.reciprocal_approx_fast(rz[:], zs[:])
        nc.vector.tensor_mul(
            EZ[:, c], Ech[:],
            rz[:].unsqueeze(1).broadcast_to([128, O, BL]))

    def agreement(it, accumulate):
        """a[b,o,i] = sum_{s,w} W x v via z-matmuls + DVE zx/tree -> A1.

        Also emits the per-chunk softmax for the NEXT iteration so it
        overlaps the remaining agreement chunks."""
        with tc.tile_pool(name=f"zps{it}", space="PSUM", bufs=2) as zp:
            for c in range(C9):
                zsb = WK.tile([128, O, S, BL], BF16, name=f"zsb{it}_{c}",
                              tag="zsb")
                for gi, (o0, on) in enumerate(((0, 4), (4, 4), (8, 2))):
                    zq = zp.tile([128, 4, S, BL], F32,
                                 name=f"zq{it}_{c}_{gi}", tag="zq")
                    for oo in range(on):
                        o = o0 + oo
                        nc.tensor.matmul(
                            zq[:, oo, :, :],
                            W2[:, o, 128 * c:128 * (c + 1)],
                            V8D[:, o, :, :],
                        )
                    nc.scalar.copy(zsb[:, o0:o0 + on, :, :], zq[:, 0:on, :, :])
                zx = WK1.tile([128, O, S, BL], BF16, name=f"zx{it}_{c}",
                              tag="zx")
                nc.vector.tensor_mul(
                    zx[:], zsb[:],
                    XB[:, c].unsqueeze(1).broadcast_to([128, O, S, BL]))
                t4 = WK1.tile([128, O, 4, BL], BF16, name=f"t4_{it}_{c}",
                              tag="t4")
                nc.vector.tensor_add(t4[:], zx[:, :, 0:4, :],
                                     zx[:, :, 4:8, :])
                t2 = WK1.tile([128, O, 2, BL], BF16, name=f"t2_{it}_{c}",
                              tag="t2")
                nc.vector.tensor_add(t2[:], t4[:, :, 0:2, :],
                                     t4[:, :, 2:4, :])
                if not accumulate:
                    nc.vector.tensor_add(A1[:, c], t2[:, :, 0, :],
                                         t2[:, :, 1, :])
                else:
                    a2c = WK1.tile([128, O, BL], F32, name=f"a2c_{c}",
                                   tag="a2c")
                    nc.vector.tensor_add(a2c[:], t2[:, :, 0, :],
                                         t2[:, :, 1, :])
                    nc.vector.tensor_add(A1[:, c], A1[:, c], a2c[:])
                softmax_chunk(c, it)

    def s_iter1():
        """s1 via lhsT=XB (c = 1/10 folded into squash scale)."""
        with tc.tile_pool(name="sps1", space="PSUM", bufs=1) as sp1p:
            sp1 = sp1p.tile([64, O, OW], F32, name="sp1")
            for c in range(C9):
                for s in range(S):
                    q, se = s % 2, s // 2
                    nc.tensor.matmul(
                        sp1[:],
                        XB[:, c, s, :],
                        WSWP[:, c, :, q, 32 * se:32 * se + 16],
                        start=(c == 0 and s == 0),
                        stop=(c == C9 - 1 and s == S - 1),
                    )
            s_bT = SM.tile([64, O, OW], F32, name="s_bT1", tag="s_bT1")
            nc.scalar.copy(s_bT[:], sp1[:])
        return s_bT

    def s_pass(it, o0, on, ytag):
        """One o-pass of blocked s-matmuls; returns s_part [16, on, BL].

        PSUM [128, 2*on, 256]: slice j=(oo,q) holds, at rows
        32se+[0:16] and cols 64se+[0:64], the partial sums for s=2se+q.
        Each bank (= one oo) is a single accumulation group."""
        with tc.tile_pool(name=f"sp_{it}_{o0}", space="PSUM", bufs=1) as pp:
            Pp = pp.tile([128, 2 * on, 4 * BL], F32, name=f"P_{it}_{o0}")
            for c in range(C9):
                y = WK.tile([128, on, S, BL], BF16,
                            name=f"y{it}_{o0}_{c}", tag=ytag)
                nc.vector.tensor_mul(
                    y[:],
                    XB[:, c].unsqueeze(1).broadcast_to([128, on, S, BL]),
                    EZ[:, c, o0:o0 + on].unsqueeze(2)
                    .broadcast_to([128, on, S, BL]))
                for oo in range(on):
                    yq = y[:, oo].rearrange("p (se q) b -> p q se b", q=2)
                    for q in range(2):
                        nc.tensor.matmul(
                            Pp[:, 2 * oo + q, :],
                            WSWP[:, c, o0 + oo, q, :],
                            yq[:, q],
                            start=(c == 0 and q == 0),
                            stop=(c == C9 - 1 and q == 1),
                        )
            # diagonal extraction: s[w,b] = sum_se P[32se+w, (se,b)] (per q)
            # (DVE may read only one PSUM operand; stage T1/T3 via scalar)
            g1 = SM.tile([16, 2 * on, BL], F32, name=f"g1_{it}_{o0}",
                         tag="g1")
            nc.scalar.copy(g1[:], Pp[32:48, :, BL:2 * BL])
            g3 = SM.tile([16, 2 * on, BL], F32, name=f"g3_{it}_{o0}",
                         tag="g3")
            nc.scalar.copy(g3[:], Pp[96:112, :, 3 * BL:4 * BL])
            h1 = SM.tile([16, 2 * on, BL], F32, name=f"h1_{it}_{o0}",
                         tag="h1")
            nc.vector.tensor_add(h1[:], Pp[0:16, :, 0:BL], g1[:])
            h2 = SM.tile([16, 2 * on, BL], F32, name=f"h2_{it}_{o0}",
                         tag="h2")
            nc.vector.tensor_add(h2[:], Pp[64:80, :, 2 * BL:3 * BL], g3[:])
            sh = SM.tile([16, on, 2, BL], F32, name=f"sh_{it}_{o0}",
                         tag="sh")
            nc.vector.tensor_add(
                sh[:], h1[:].rearrange("p (o q) b -> p o q b", q=2),
                h2[:].rearrange("p (o q) b -> p o q b", q=2))
            sp = SM.tile([16, on, BL], F32, name=f"s8_{it}_{o0}",
                         tag=f"s8_{o0}")
            nc.vector.tensor_add(sp[:], sh[:, :, 0, :], sh[:, :, 1, :])
        return sp

    def s_iter23(it):
        s8 = s_pass(it, 0, 4, "y4a")
        s4 = s_pass(it, 4, 4, "y4b")
        s2 = s_pass(it, 8, 2, "y2")
        with tc.tile_pool(name=f"tps{it}", space="PSUM", bufs=1) as tpp:
            tsp = tpp.tile([64, O, OW], F32, name=f"tsp{it}")
            for oo in range(4):
                nc.tensor.transpose(tsp[:, oo, :], s8[:, oo, :], ID16[:])
            for oo in range(4):
                nc.tensor.transpose(tsp[:, 4 + oo, :], s4[:, oo, :], ID16[:])
            for oo in range(2):
                nc.tensor.transpose(tsp[:, 8 + oo, :], s2[:, oo, :], ID16[:])
            s_bT = SM.tile([64, O, OW], F32, name=f"s_bT{it}",
                           tag="s_bT23")
            nc.scalar.copy(s_bT[:], tsp[:])
        return s_bT

    # ---- iteration 1 ------------------------------------------------------
    s_bT = s_iter1()
    v1 = squash(s_bT, 0.1, 1)
    build_v8d(v1, 1)
    agreement(1, accumulate=False)

    # ---- iteration 2 ------------------------------------------------------
    s_bT = s_iter23(2)
    v2 = squash(s_bT, 1.0, 2)
    build_v8d(v2, 2)
    agreement(2, accumulate=True)

    # ---- iteration 3 ------------------------------------------------------
    s_bT = s_iter23(3)
    v3 = squash(s_bT, 1.0, 3)
    nc.sync.dma_start(t["v"][:], v3[:])


def _build_nc():
    nc = bacc.Bacc("TRN2", target_bir_lowering=False)
    t = {
        "xb": nc.dram_tensor("xb", [128, C9, S, BL], BF16,
                             kind="ExternalInput"),
        "wswp": nc.dram_tensor("wswp", [128, C9, O, 2, 128], BF16,
                               kind="ExternalInput"),
        "w2": nc.dram_tensor("w2", [128, O, I], BF16, kind="ExternalInput"),
        "id64": nc.dram_tensor("id64", [64, 64], F32, kind="ExternalInput"),
        "id16": nc.dram_tensor("id16", [16, 16], F32, kind="ExternalInput"),
        "v": nc.dram_tensor("v", [BL, O, OW], F32, kind="ExternalOutput"),
    }
    from contextlib import ExitStack
    with tile.TileContext(nc) as tc, ExitStack() as ctx:
        _emit(nc, tc, t, ctx)
    nc.finalize()
    return nc


_NC_CACHE = {}


def _get_nc():
    if "nc" not in _NC_CACHE:
        _NC_CACHE["nc"] = _build_nc()
    return _NC_CACHE["nc"]


# ---------------------------------------------------------------------------
# host side
# ---------------------------------------------------------------------------

def _host_layouts(x, W):
    xr = np.ascontiguousarray(np.asarray(x, np.float32)).reshape(B, I, S)
    W0 = np.asarray(W, np.float32)[0]                     # [O, I, OW, S]

    xbs = []
    for k in range(NCORES):
        xc = xr[k * BL:(k + 1) * BL]
        tmp = xc.transpose(1, 2, 0)                       # [I, S, BL]
        xb = tmp.reshape(C9, 128, S, BL).transpose(1, 0, 2, 3)
        xbs.append(np.ascontiguousarray(xb).astype(BF))

    # WSWP[p, c, o, q, 32se+w] = W0[o, 128c+p, w, 2se+q]; rest zero
    warr = W0.transpose(1, 0, 3, 2)                       # [I, O, S, OW]
    warr = warr.reshape(C9, 128, O, S, OW).transpose(1, 0, 2, 3, 4)
    wswp = np.zeros((128, C9, O, 2, 4, 32), np.float32)
    for q in range(2):
        for se in range(4):
            wswp[:, :, :, q, se, 0:OW] = warr[:, :, :, 2 * se + q, :]
    wswp = wswp.reshape(128, C9, O, 2, 128).astype(BF)

    # W2[16s+w, o, i] = W0[o, i, w, s]
    w2 = np.zeros((128, O, I), np.float32)
    for s in range(S):
        w2[16 * s:16 * s + OW] = W0[:, :, :, s].transpose(2, 0, 1)
    w2 = w2.astype(BF)

    id64 = np.eye(64, dtype=np.float32)
    id16 = np.eye(16, dtype=np.float32)
    return xbs, wswp, w2, id64, id16


def _in_maps(x, W):
    xbs, wswp, w2, id64, id16 = _host_layouts(x, W)
    return [
        {"xb": xbs[k], "wswp": wswp, "w2": w2, "id64": id64, "id16": id16}
        for k in range(NCORES)
    ]


def kernel(x, W):
    nc = _get_nc()
    in_maps = _in_maps(x, W)
    res = run_bass_kernel_spmd(nc, in_maps, core_ids=list(range(NCORES)))
    out = np.concatenate([r["v"] for r in res.results], axis=0)
    return np.ascontiguousarray(out.astype(np.float32))


if __name__ == "__main__":
    rng = np.random.default_rng(0)
    x = rng.standard_normal((B, 32, S, 6, 6), dtype=np.float32)
    W = rng.uniform(-1, 1, (1, O, I, OW, S)).astype(np.float32) / np.sqrt(S)
    v = kernel(x, W)
    print("out", v.shape, v.dtype, float(np.abs(v).max()))
